# revision 16
# baseline (speedup 1.0000x reference)
"""Trainium2 Bass kernel for nn_CentralityEncoding (8 NeuronCores, SPMD).

Math (reference):
    out = x + z_in[min(in_deg,511)] + z_out[min(out_deg,511)]
        + sigmoid(cent @ W_cent + b_cent) + sigmoid(vor @ W_vor + b_vor)
        + segment_sum(edge_attr @ W_sum + b_sum, src)

Device computes the E-level work (1.6M-edge segment sum + the [N,128]@
[128,256] projection); the N-level pointwise terms are folded on host:
    x_eff = x + z_in[ideg] + z_out[odeg] + sigmoid(cent*Wc+bc)
          + sigmoid(vor*Wv+bv) + out_deg_raw*b_sum        (bf16)
    out   = x_eff + segment_sum(edge_fp8, src) @ (W_sum*step)

Sharding/layout (v3 — DoubleRow quad matmuls):
  Nodes sorted by out-degree (desc); rank r -> pair-octet j = r//1024
  (1024 ranks = 8 cores x 128-node "pairs"); snake block assignment
  inside each octet keeps all 8 cores' programs identical with
  T_j = ceil(deg_rank[1024j]/2) matmul steps per local pair j.

  Pairs are grouped 4-per-quad. ONE DoubleRow matmul per quad step:
      lhsT = [I128 | I128]  (fp8, [128,2,128], CONSTANT -> LDWEIGHTS
             never changes and is fully hidden),
      rhs  = [128, 2, 512]: plane p, group g holds edge #(2t+p) of every
             node of pair 4q+g  (16 KB fp8 = 8 edge-tiles per matmul),
      out  = PSUM [128, 512] f32: column group g accumulates pair 4q+g.
  Measured on hw: 216 ns per quad matmul = 27 ns per edge-tile, 2x the
  per-tile rate of the v1 data-stationary formulation (and the one-hot
  z-matmuls + their 6.4 MB of DMA are gone entirely).

  Aggregates are node-major, so phase 2 runs per pair:
      PE transpose [128,128] (measured ~60-90 ns in-stream)
      -> aggT bf16 -> pp = aggT.T @ W_sum' + I.T @ x_eff (PSUM f32)
      -> ACT Copy evacuates pp to the staged bf16 output.
  Edge features ship as fp8 e4m3 with sigma-delta (error-feedback)
  encoding per (node, feature) stream, exactly as v1.
"""

import numpy as np
import ml_dtypes

import bass_rust
import concourse.bass as bass
import concourse.mybir as mybir
import concourse.tile as tile
from concourse.bass_utils import run_bass_kernel_spmd
from concourse.vector_clock import ScopedClock

# ----------------------------------------------------------------------------
# Problem constants (hardcoded per the harness contract).
N_NODES = 50000
N_EDGES = 1600000
NODE_DIM = 256
EDGE_DIM = 128
MAX_DEG = 512
N_CORES = 8
P = 128
NPAIR = 49                      # 128-node pairs per core
NQUAD = 12                      # quads of 4 pairs; pair 48 is the tail
NPAD = NPAIR * P                # 6272 node slots per core
NRANK = NPAIR * 1024            # 50176 ranks incl. dummies
SPC = 8                         # quad-steps per DMA piece
F32 = mybir.dt.float32
BF16 = mybir.dt.bfloat16
FP8 = mybir.dt.float8e4
FP8NP = mybir.dt.np(FP8)
BF16NP = mybir.dt.np(BF16)
DR = mybir.MatmulPerfMode.DoubleRow


# ----------------------------------------------------------------------------
# Workarounds for this container's walrus build, which rejects any
# instruction carrying more than ONE semaphore wait ("Too many sync wait
# commands", CoreV3GenImpl setupSyncWait).

_orig_commit = tile.TileContext._commit_instruction


def _commit_split_waits(self, inst, lazy_reg_writes=True):
    si = getattr(inst, "sync_info", None)
    if si is not None and si.on_wait and len(si.on_wait) > 1:
        waits = list(si.on_wait)
        for w in waits[:-1]:
            nop = mybir.InstNoOp(
                name=self.nc.get_next_instruction_name(),
                sync_info=mybir.SyncInfo(on_wait=[w], on_update=[]),
                bass_nofuse=True,
                engine=inst.engine,
            )
            _orig_commit(self, nop, lazy_reg_writes)
        inst.sync_info = mybir.SyncInfo(
            on_wait=[waits[-1]], on_update=list(si.on_update)
        )
    return _orig_commit(self, inst, lazy_reg_writes)


tile.TileContext._commit_instruction = _commit_split_waits


def _patched_drain_and_barrier(self, tick_clock, wait_clock):
    nc = self.nc
    collector = nc.sync.nop(nofuse=True)
    wait_clock.add_sem_waits(
        collector.ins, ScopedClock({None: tick_clock.global_clock})
    )
    si = collector.ins.sync_info
    waits = list(si.on_wait) if si is not None else []
    if waits:
        collector.ins.sync_info = bass_rust.SyncInfo(
            on_wait=[waits[0]], on_update=[]
        )
        for w in waits[1:]:
            nop = nc.sync.nop(nofuse=True)
            nop.ins.sync_info = bass_rust.SyncInfo(on_wait=[w], on_update=[])
    nc.sync.drain()
    nc.all_engine_barrier()
    assert self.sems is not None
    popped = nc._tile_sem_poison_stack.pop()
    assert popped is self._sem_poison
    nc.clear_and_free_semaphores(list(self.sems.allocated().values()))
    nc.all_engine_barrier()


tile.TileContext._drain_and_barrier = _patched_drain_and_barrier


# ----------------------------------------------------------------------------
def build_program(steps: tuple, t_tail: int) -> bass.Bass:
    steps = list(steps)            # 12 quad step counts
    sb = np.concatenate([[0], np.cumsum(steps)]).astype(int)
    TOT = int(sb[-1])              # total quad-steps
    nc = bass.Bass()

    aq_d = nc.declare_dram_parameter("aq", [P, TOT * 1024], FP8, isOutput=False)
    at_d = nc.declare_dram_parameter("at", [P, t_tail * 256], FP8, isOutput=False)
    sel_d = nc.declare_dram_parameter("sel", [P, 2 * P], FP8, isOutput=False)
    ident_d = nc.declare_dram_parameter("ident", [P, P], BF16, isOutput=False)
    identx_d = nc.declare_dram_parameter("identx", [P, P], BF16, isOutput=False)
    wsum_d = nc.declare_dram_parameter("W_sum", [EDGE_DIM, NODE_DIM], BF16, isOutput=False)
    x_d = nc.declare_dram_parameter("x", [P, NPAIR * NODE_DIM], FP8, isOutput=False)
    out_d = nc.declare_dram_parameter("out", [P, NPAIR * NODE_DIM], BF16, isOutput=True)

    copyf = mybir.ActivationFunctionType.Copy

    # schedule: pieces of SPC consecutive quad-steps
    sched = [(q, t) for q in range(NQUAD) for t in range(steps[q])]
    pieces = [sched[i:i + SPC] for i in range(0, len(sched), SPC)]

    with tile.TileContext(nc) as tc:
        with (
            tc.tile_pool(name="const", bufs=1) as const,
            tc.tile_pool(name="apool", bufs=12) as apool,
            tc.tile_pool(name="aggp", bufs=3) as aggp,
            tc.tile_pool(name="atp", bufs=4) as atp,
            tc.tile_pool(name="psq", bufs=4, space="PSUM") as psq,
            tc.tile_pool(name="ptp", bufs=2, space="PSUM") as ptp,
            tc.tile_pool(name="ppp", bufs=2, space="PSUM") as ppp,
        ):
            # one-time constants
            selb = const.tile([P, 2, P], FP8, tag="selb")
            nc.scalar.dma_start(out=selb[:], in_=sel_d[:])
            ident = const.tile([P, P], BF16, tag="ident")
            nc.scalar.dma_start(out=ident[:], in_=ident_d[:])
            identx = const.tile([P, P], BF16, tag="identx")
            nc.scalar.dma_start(out=identx[:], in_=identx_d[:])
            wsum_b = const.tile([EDGE_DIM, NODE_DIM], BF16, tag="wsum_b")
            nc.scalar.dma_start(out=wsum_b[:], in_=wsum_d[:])
            # x halves are DMA'd late (below) so the edge stream owns the
            # HBM port while the pipeline fills
            x_sb = const.tile([P, NPAIR * NODE_DIM], FP8, tag="x_sb")
            o_sb = const.tile([P, NPAIR * NODE_DIM], BF16, tag="o_sb")
            tailb = const.tile([P, t_tail, 2, P], FP8, tag="tailb")
            nc.gpsimd.dma_start(out=tailb[:], in_=at_d[:])

            psq_t: dict[int, object] = {}

            # HAM warmup: keep PE busy while the first pieces stream in, so
            # the real matmuls start at K=8/8 (2.4 GHz) instead of cold
            warm = ppp.tile([P, NODE_DIM], F32, space="PSUM", name="pp", tag="pp")
            for _ in range(40):
                nc.tensor.matmul(out=warm[:, :64], lhsT=selb[:],
                                 rhs=selb[:, :, :64],
                                 perf_mode=DR, skip_group_check=True)

            def post_pairs(js, agg_sb):
                # stage 1: PE transposes (pipelined with DVE copies)
                aggts = []
                for g, j in enumerate(js):
                    pt = ptp.tile([P, P], BF16, space="PSUM")
                    nc.tensor.transpose(pt[:], agg_sb[:, g * P:(g + 1) * P],
                                        ident[:])
                    aggt = atp.tile([P, P], BF16)
                    nc.vector.tensor_copy(aggt[:], pt[:])
                    aggts.append(aggt)
                return aggts

            def post_quad(q):
                # per-pair casts so the first transpose starts ~250ns after
                # the quad's stop matmul instead of waiting a 700ns full-quad
                # cast
                ps = psq_t.pop(q)
                agg_sb = aggp.tile([P, 4 * P], BF16)
                aggts = []
                for g in range(4):
                    csl = slice(g * P, (g + 1) * P)
                    nc.vector.tensor_copy(agg_sb[:, csl], ps[:, csl])
                    pt = ptp.tile([P, P], BF16, space="PSUM")
                    nc.tensor.transpose(pt[:], agg_sb[:, csl], ident[:])
                    aggt = atp.tile([P, P], BF16)
                    nc.vector.tensor_copy(aggt[:], pt[:])
                    aggts.append(aggt)
                for g in range(4):
                    j = 4 * q + g
                    pp = ppp.tile([P, NODE_DIM], F32, space="PSUM", name="pp", tag="pp")
                    nc.tensor.matmul(out=pp[:], lhsT=aggts[g][:], rhs=wsum_b[:],
                                     start=True, stop=False, skip_group_check=True)
                    osl = slice(j * NODE_DIM, (j + 1) * NODE_DIM)
                    nc.tensor.matmul(out=pp[:], lhsT=identx[:], rhs=x_sb[:, osl],
                                     start=False, stop=True, skip_group_check=True)
                    nc.scalar.activation(out=o_sb[:, osl], in_=pp[:], func=copyf)

            # store finished output columns, batched
            def store(lo_pair, hi_pair):
                lo, hi = lo_pair * NODE_DIM, hi_pair * NODE_DIM
                nc.scalar.dma_start(out=out_d[:, lo:hi], in_=o_sb[:, lo:hi])

            def run_tail():
                # tail pair (j=48): DoubleRow pair matmuls, run early so the
                # kernel end is just post_quad(11) + final store
                pst = psq.tile([P, 4 * 128], F32, space="PSUM", name="psq", tag="psq")
                for t in range(t_tail):
                    nc.tensor.matmul(out=pst[:, :P], lhsT=selb[:],
                                     rhs=tailb[:, t, :, :],
                                     start=(t == 0), stop=(t == t_tail - 1),
                                     perf_mode=DR, skip_group_check=True)
                agg_sb = aggp.tile([P, 4 * P], BF16)
                nc.vector.tensor_copy(agg_sb[:, :P], pst[:, :P])
                aggts = post_pairs([48], agg_sb)
                pp = ppp.tile([P, NODE_DIM], F32, space="PSUM", name="pp", tag="pp")
                nc.tensor.matmul(out=pp[:], lhsT=aggts[0][:], rhs=wsum_b[:],
                                 start=True, stop=False, skip_group_check=True)
                osl = slice(48 * NODE_DIM, 49 * NODE_DIM)
                nc.tensor.matmul(out=pp[:], lhsT=identx[:], rhs=x_sb[:, osl],
                                 start=False, stop=True, skip_group_check=True)
                nc.scalar.activation(out=o_sb[:, osl], in_=pp[:], func=copyf)

            posted = [False] * NQUAD
            XSPLIT = 24 * NODE_DIM

            for pi, piece in enumerate(pieces):
                n = len(piece)
                pt8 = apool.tile([P, SPC, 2, 512], FP8)
                lo = sb[piece[0][0]] + piece[0][1]
                nc.sync.dma_start(out=pt8[:, :n, :, :],
                                  in_=aq_d[:, lo * 1024:(lo + n) * 1024])
                if pi == 2:
                    nc.scalar.dma_start(out=x_sb[:, :XSPLIT], in_=x_d[:, :XSPLIT])
                elif pi == 3:
                    nc.scalar.dma_start(out=x_sb[:, 48 * NODE_DIM:],
                                        in_=x_d[:, 48 * NODE_DIM:])
                elif pi == 8:
                    nc.scalar.dma_start(out=x_sb[:, XSPLIT:48 * NODE_DIM],
                                        in_=x_d[:, XSPLIT:48 * NODE_DIM])
                for si, (q, t) in enumerate(piece):
                    if t == 0:
                        psq_t[q] = psq.tile([P, 4 * 128], F32, space="PSUM", name="psq", tag="psq")
                    nc.tensor.matmul(out=psq_t[q][:], lhsT=selb[:],
                                     rhs=pt8[:, si, :, :],
                                     start=(t == 0), stop=(t == steps[q] - 1),
                                     perf_mode=DR, skip_group_check=True)
                    # lag quad post-processing ~4 steps into the next quad
                    if q > 0 and not posted[q - 1] and t >= min(2, steps[q] - 1):
                        post_quad(q - 1)
                        posted[q - 1] = True
                        if q - 1 == 2:
                            run_tail()
                        elif q - 1 == 3:
                            store(0, 12)
                        elif q - 1 == 6:
                            store(12, 24)
                        elif q - 1 == 9:
                            store(24, 36)
                        elif q - 1 == 10:
                            store(36, 44)

            post_quad(NQUAD - 1)
            store(44, 49)
    return nc


# ----------------------------------------------------------------------------
def prepare_inputs(x, edge_index, edge_attr, voronoi_values, centralities,
                   z_in, z_out, W_cent, b_cent, W_vor, b_vor, W_sum, b_sum):
    """Host-side sharding: degree-sort nodes into 128-node pairs grouped
    4-per-quad, place each node's edges into fixed (step, plane, group)
    slots; sigma-delta-quantize edge features to fp8; fold all node-level
    pointwise terms into x_eff.  Returns (in_maps, build_key, asm)."""
    src = np.asarray(edge_index[0], dtype=np.int64)
    dst = np.asarray(edge_index[1], dtype=np.int64)
    edge_attr = np.asarray(edge_attr, dtype=np.float32)
    x = np.asarray(x, dtype=np.float32)

    deg = np.bincount(src, minlength=N_NODES).astype(np.int64)
    in_deg_raw = np.bincount(dst, minlength=N_NODES).astype(np.int64)
    in_deg = np.minimum(in_deg_raw, MAX_DEG - 1)
    out_deg = np.minimum(deg, MAX_DEG - 1)

    # fp8 sigma-delta encoding scale; folds into W_sum
    sd = float(edge_attr[::17].std()) or 1.0
    step = sd / 32.0

    # degree-sorted ranking
    order_nodes = np.argsort(-deg, kind="stable")
    rank_of_node = np.empty(N_NODES, dtype=np.int64)
    rank_of_node[order_nodes] = np.arange(N_NODES)
    deg_rank = np.zeros(NRANK, dtype=np.int64)
    deg_rank[:N_NODES] = deg[order_nodes]

    # per-pair-octet matmul step counts (identical across cores)
    T = np.maximum(1, (deg_rank[::1024] + 1) // 2).astype(np.int64)  # [49]
    steps = tuple(int(max(T[4 * q:4 * q + 4].max(), 1)) for q in range(NQUAD))
    t_tail = int(T[48])
    sb = np.concatenate([[0], np.cumsum(steps)]).astype(np.int64)
    TOT = int(sb[-1])

    # per-edge slot placement
    rho = rank_of_node[src]
    eorder = np.argsort(rho, kind="stable")
    rhos = rho[eorder]
    st_rank = np.concatenate([[0], np.cumsum(deg_rank)])[:-1]
    i_e = np.arange(N_EDGES, dtype=np.int64) - st_rank[rhos]

    j_e = rhos // 1024
    w_e = rhos % 1024
    blk_e = w_e // P
    q_e = w_e % P
    c_e = np.where(j_e % 2 == 0, blk_e, 7 - blk_e)
    t_e = i_e // 2
    p_e = i_e % 2
    main = j_e < 4 * NQUAD
    # 128-col block index within each core's buffers
    gs_e = sb[np.minimum(j_e // 4, NQUAD - 1)] + t_e
    blk128_main = (c_e * P + q_e) * (TOT * 8) + gs_e * 8 + p_e * 4 + (j_e % 4)
    blk128_tail = (c_e * P + q_e) * (t_tail * 2) + t_e * 2 + p_e

    # Sigma-delta fp8 encoding, per (node, feature) stream.
    cs = (edge_attr[eorder] * (1.0 / step)).astype(np.float32)
    q8s = np.empty((N_EDGES, EDGE_DIM), dtype=FP8NP)
    resid = np.zeros((NRANK, EDGE_DIM), dtype=np.float32)
    maxd = int(deg_rank.max())
    for i in range(maxd):
        nodes_i = np.nonzero(deg_rank > i)[0]
        idx = st_rank[nodes_i] + i
        t = cs[idx] + resid[nodes_i]
        q = t.astype(FP8NP)
        q8s[idx] = q
        resid[nodes_i] = t - q.astype(np.float32)

    a_main = np.zeros((N_CORES * P * TOT * 8, EDGE_DIM), dtype=FP8NP)
    a_main[blk128_main[main]] = q8s[main]
    a_main = a_main.reshape(N_CORES, P, TOT * 1024)
    a_tail = np.zeros((N_CORES * P * t_tail * 2, EDGE_DIM), dtype=FP8NP)
    a_tail[blk128_tail[~main]] = q8s[~main]
    a_tail = a_tail.reshape(N_CORES, P, t_tail * 256)

    # node-rank -> (core, pair, row) map for x/out permutation
    rr = np.arange(NRANK)
    j_r = rr // 1024
    w_r = rr % 1024
    blk_r = w_r // P
    c_r = np.where(j_r % 2 == 0, blk_r, 7 - blk_r)
    slot_r = j_r * P + (w_r % P)       # local slot 0..NPAD-1 on core c_r
    rank_at = np.empty((N_CORES, NPAD), dtype=np.int64)
    rank_at[c_r, slot_r] = rr

    # x_eff: all node-level pointwise terms folded (f32 on host)
    W_cent = np.asarray(W_cent, dtype=np.float32).reshape(1, NODE_DIM)
    W_vor = np.asarray(W_vor, dtype=np.float32).reshape(1, NODE_DIM)
    b_cent = np.asarray(b_cent, dtype=np.float32).reshape(1, NODE_DIM)
    b_vor = np.asarray(b_vor, dtype=np.float32).reshape(1, NODE_DIM)
    b_sum = np.asarray(b_sum, dtype=np.float32).reshape(1, NODE_DIM)
    cent = np.asarray(centralities, dtype=np.float32).reshape(N_NODES, 1)
    vor = np.asarray(voronoi_values, dtype=np.float32).reshape(N_NODES, 1)

    def sigm(v):
        return 1.0 / (1.0 + np.exp(-v))

    x_eff = (x
             + np.asarray(z_in, dtype=np.float32)[in_deg]
             + np.asarray(z_out, dtype=np.float32)[out_deg]
             + sigm(cent @ W_cent + b_cent)
             + sigm(vor @ W_vor + b_vor)
             + deg.astype(np.float32)[:, None] * b_sum)

    XSCALE = 16.0                      # fp8 x shipped scaled; 1/XSCALE is
    padded = np.zeros((NRANK, NODE_DIM), dtype=np.float32)   # folded into identx
    padded[:N_NODES] = x_eff[order_nodes]
    x_pc = padded[rank_at]             # [cores, NPAD, 256]
    x_pm = np.ascontiguousarray(
        x_pc.reshape(N_CORES, NPAIR, P, NODE_DIM).transpose(0, 2, 1, 3)
        .reshape(N_CORES, P, NPAIR * NODE_DIM) * XSCALE).astype(FP8NP)

    eye = np.eye(P, dtype=np.float32)
    sel = np.concatenate([eye, eye], axis=1).astype(FP8NP)
    W_sum_eff = (np.asarray(W_sum, dtype=np.float32) * step).astype(BF16NP)

    in_maps = []
    for c in range(N_CORES):
        in_maps.append({
            "aq": a_main[c],
            "at": a_tail[c],
            "sel": sel,
            "ident": eye.astype(BF16NP),
            "identx": (eye / XSCALE).astype(BF16NP),
            "W_sum": W_sum_eff,
            "x": x_pm[c],
        })
    asm = {"order_nodes": order_nodes, "c_r": c_r, "slot_r": slot_r}
    key = (steps, t_tail)
    return in_maps, key, asm


def assemble_output(results, asm):
    """results: list of per-core 'out' arrays [P, NPAIR*NODE_DIM]."""
    outs = np.stack([
        np.asarray(results[c], dtype=np.float32)
        .reshape(P, NPAIR, NODE_DIM).transpose(1, 0, 2).reshape(NPAD, NODE_DIM)
        for c in range(N_CORES)
    ])
    out_sorted = outs[asm["c_r"], asm["slot_r"]]      # [NRANK, 256]
    out_full = np.empty((N_NODES, NODE_DIM), dtype=np.float32)
    out_full[asm["order_nodes"]] = out_sorted[:N_NODES]
    return out_full


_PROGRAM_CACHE: dict[tuple, bass.Bass] = {}


def kernel(**inputs) -> np.ndarray:
    in_maps, key, asm = prepare_inputs(**inputs)
    nc = _PROGRAM_CACHE.get(key)
    if nc is None:
        nc = build_program(*key)
        _PROGRAM_CACHE[key] = nc
    res = None
    for attempt in range(3):
        try:
            res = run_bass_kernel_spmd(nc, in_maps, core_ids=list(range(N_CORES)))
            break
        except Exception:
            # axon transiently reports "accelerator device unrecoverable";
            # a clean retry succeeds
            if attempt == 2:
                raise
    return assemble_output([res.results[i]["out"] for i in range(N_CORES)], asm)
